# revision 48
# baseline (speedup 1.0000x reference)
"""Trainium2 Bass kernel for MFVIConstituency mean-field iterations.

Per batch b (one NeuronCore each, 8 total):
    q = s_con;  repeat 3x:  q[i,j] = s_con[i,j] + sum_k sig(q)[j,k] * sb[i,j,k]
    out = sigmoid(q)
where sb = s_bin * mask2o, mask2o[i,j,k] = mask[i,j] & (i!=k) & (j!=k).

Strategy: the contraction sum_k sig(q)[j,k]*sb[i,j,k] is, for each fixed j, a
matvec with a j-dependent matrix -- so it runs on the otherwise-idle PE array
as per-j matvec groups: stationary = sb[:,j,:]^T (fp16, k on partitions, i on
the stationary free dim), moving = one column of sig(q)^T, accumulating the
two k-chunks (128 and 64+1) into PSUM in fp32.  The s_con add is folded in as
a 193rd contraction row whose moving value is 1.0 and whose stationary row
holds s_con[:,j].

The 14.2MB weight cache streams in over all three DMA-capable queues
(SP/ACT/Pool) as interleaved j-chunks; iteration 1 consumes them as they
land (its PSUM groups are per-j: start/stop adjacent, two groups per j for
the two i-chunks).  Iteration boundaries: two full-size ACT sigmoids (PSUM
f32 -> SBUF fp16; matmul clusters dispatch instantaneously in-model, so
halving buys no overlap and only serializes ACT), PE transposes via an
identity matrix (fp16 PSUM), copy-backs split DVE/ACT and ordered so the
next iteration's first columns unblock first.  The sigmoid act table is
preloaded by a warmup activation that hides between the DMA ladder and
boundary 1.

Host (numpy) does input prep only: masking, fp16 cast, [k,j,i] layout
packing, and the iteration-1 sigmoid of s_con (an input-operand transform,
same as the previous kernel).  The identity, the iteration-1 sigmoid
operands, and the s_con row are piggybacked onto the two weight tensors,
so the whole problem moves in 2 DRAM tensors / 48 chunked transfers.
"""

import numpy as np

S = 192
B = 8
P = 128
SS = S * S            # 36864
H0 = P + S            # c0 header: identity(128) | st0(192)
H1 = S                # c1x header: st1x(192)
C0W = H0 + SS
C1W = H1 + SS
NJC = 24              # DMA j-chunks (8 j per chunk)
JC = S // NJC

_CACHE = {}


def _build_program():
    import concourse.tile as tile
    from concourse import mybir, bacc
    from contextlib import ExitStack

    f32, f16 = mybir.dt.float32, mybir.dt.float16
    Sig = mybir.ActivationFunctionType.Sigmoid

    nc = bacc.Bacc("TRN2", target_bir_lowering=False, debug=False, num_devices=B)

    c0_d = nc.dram_tensor("c0", [P, C0W], f16, kind="ExternalInput")
    c1x_d = nc.dram_tensor("c1x", [65, C1W], f16, kind="ExternalInput")
    qout_d = nc.dram_tensor("qout", [S, S], f32, kind="ExternalOutput")

    with tile.TileContext(nc) as tc, ExitStack() as ctx:
        cache_p = ctx.enter_context(tc.tile_pool(name="cache", bufs=1))
        sig_p = ctx.enter_context(tc.tile_pool(name="sig", bufs=2))
        out_p = ctx.enter_context(tc.tile_pool(name="out", bufs=1))
        warm_p = ctx.enter_context(tc.tile_pool(name="warm", bufs=1))
        qp_p = ctx.enter_context(tc.tile_pool(name="qp", bufs=2, space="PSUM"))
        tp_p = ctx.enter_context(tc.tile_pool(name="tp", bufs=1, space="PSUM"))

        SBT0 = cache_p.tile([P, C0W], f16, tag="sbt0")
        SBT1x = cache_p.tile([65, C1W], f16, tag="sbt1")
        ID = SBT0[:, 0:P]                   # eye(128)
        SIGT0 = SBT0[:, P:H0]               # sig(q)^T rows 0:128, col j
        SIGT1x = SBT1x[:, 0:H1]             # rows 128:192 + ones row

        def w0(j):                          # stationary slice base in SBT0/SBT1x
            return H0 + j * S

        def w1(j):
            return H1 + j * S

        warm = warm_p.tile([1, 2], f16, tag="warm")
        wsig = warm_p.tile([1, 2], f16, tag="wsig")

        # chunked round-robin DMA across the 3 queues (2 concurrent in flight);
        # chunk 0 carries the identity + iteration-1 sigmoid operands up front
        qs = [nc.sync, nc.scalar, nc.gpsimd]

        def dma_round(c):
            lo0 = 0 if c == 0 else H0 + c * JC * S
            lo1 = 0 if c == 0 else H1 + c * JC * S
            hi0, hi1 = H0 + (c + 1) * JC * S, H1 + (c + 1) * JC * S
            qs[(2 * c) % 3].dma_start(SBT0[:, lo0:hi0], c0_d.ap()[:, lo0:hi0])
            qs[(2 * c + 1) % 3].dma_start(SBT1x[:, lo1:hi1], c1x_d.ap()[:, lo1:hi1])

        def mm4(qp0, qp1, j):
            b0, b1 = w0(j), w1(j)
            sg0, sg1 = SIGT0[:, j:j + 1], SIGT1x[:, j:j + 1]
            nc.tensor.matmul(qp0[:, j:j + 1], SBT0[:, b0:b0 + 128], sg0,
                             start=True, stop=False, skip_group_check=True)
            nc.tensor.matmul(qp0[:, j:j + 1], SBT1x[:, b1:b1 + 128], sg1,
                             start=False, stop=True, skip_group_check=True)
            nc.tensor.matmul(qp1[:, j:j + 1], SBT0[:, b0 + 128:b0 + 192], sg0,
                             start=True, stop=False, skip_group_check=True)
            nc.tensor.matmul(qp1[:, j:j + 1], SBT1x[:, b1 + 128:b1 + 192], sg1,
                             start=False, stop=True, skip_group_check=True)

        def bnd_tiles():
            SIG0 = sig_p.tile([P, S], f16, tag="sig0")
            SIG1 = sig_p.tile([64, S], f16, tag="sig1")
            tp1 = tp_p.tile([P, P], f16, tag="tp1")
            tp2 = tp_p.tile([P, 64], f16, tag="tp2")
            tp3 = tp_p.tile([64, P], f16, tag="tp3")
            tp4 = tp_p.tile([64, 64], f16, tag="tp4")
            return SIG0, SIG1, tp1, tp2, tp3, tp4

        def bnd_lo(qp0, qp1, T):
            # sigmoid + transpose for q columns 0:128 (feeds SIGT cols 0:128);
            # copies deferred to bnd_hi so old-operand readers stay correct
            SIG0, SIG1, tp1, tp2, tp3, tp4 = T
            nc.scalar.activation(SIG0[:, 0:128], qp0[:, 0:128], Sig)
            nc.scalar.activation(SIG1[:, 0:128], qp1[:, 0:128], Sig)
            nc.tensor.transpose(tp1[:], SIG0[:, 0:128], ID)
            nc.tensor.transpose(tp2[:], SIG1[:, 0:128], ID[0:64, 0:64])

        def bnd_hi(qp0, qp1, T):
            # sigmoid + transpose for q columns 128:192 (feeds SIGT1x)
            SIG0, SIG1, tp1, tp2, tp3, tp4 = T
            nc.scalar.activation(SIG0[:, 128:192], qp0[:, 128:192], Sig)
            nc.scalar.activation(SIG1[:, 128:192], qp1[:, 128:192], Sig)
            nc.tensor.transpose(tp3[:], SIG0[:, 128:192], ID)
            nc.tensor.transpose(tp4[:], SIG1[:, 128:192], ID[0:64, 0:64])
            nc.vector.tensor_copy(SIGT0[:, 0:128], tp1[:])
            nc.vector.tensor_copy(SIGT1x[0:64, 0:128], tp3[:])
            nc.vector.tensor_copy(SIGT0[:, 128:192], tp2[:])
            nc.vector.tensor_copy(SIGT1x[0:64, 128:192], tp4[:])

        # --- iteration 1: per-j groups ride the DMA stream ------------------
        qpA0 = qp_p.tile([P, S], f32, tag="qp0")
        qpA1 = qp_p.tile([64, S], f32, tag="qp1")
        for c in range(NJC):
            dma_round(c)
        nc.vector.memset(warm[:], 0.0)
        nc.scalar.activation(wsig[:], warm[:], Sig)   # preload sigmoid table
        for j in range(S):
            mm4(qpA0, qpA1, j)
        # boundary 1: start is hard-gated (ACT busy with DMA until the ladder
        # ends), so two full-size sigmoids beat four halves; copies ordered so
        # iteration 2's first columns unblock first
        TA = bnd_tiles()
        SIG0, SIG1, tp1, tp2, tp3, tp4 = TA
        nc.scalar.activation(SIG0[:], qpA0[:], Sig)
        nc.scalar.activation(SIG1[:], qpA1[:], Sig)
        nc.tensor.transpose(tp1[:], SIG0[:, 0:128], ID)
        nc.tensor.transpose(tp3[:], SIG0[:, 128:192], ID)
        nc.tensor.transpose(tp2[:], SIG1[:, 0:128], ID[0:64, 0:64])
        nc.tensor.transpose(tp4[:], SIG1[:, 128:192], ID[0:64, 0:64])
        nc.vector.tensor_copy(SIGT0[:, 0:128], tp1[:])
        nc.vector.tensor_copy(SIGT1x[0:64, 0:128], tp3[:])
        nc.scalar.copy(SIGT0[:, 128:192], tp2[:])
        nc.vector.tensor_copy(SIGT1x[0:64, 128:192], tp4[:])

        # --- iterations 2..3 ------------------------------------------------
        # per-j groups again; the boundary's lo-half sigmoids only need q
        # columns 0:128, so the scheduler starts them while the PE finishes
        # columns 128:192 (ACT is free after the DMA ladder)
        for it in (1, 2):
            qp0 = qp_p.tile([P, S], f32, tag="qp0")
            qp1 = qp_p.tile([64, S], f32, tag="qp1")
            for j in range(S):
                mm4(qp0, qp1, j)
            if it < 2:
                T = bnd_tiles()
                SIG0b, SIG1b, btp1, btp2, btp3, btp4 = T
                nc.scalar.activation(SIG0b[:], qp0[:], Sig)
                nc.scalar.activation(SIG1b[:], qp1[:], Sig)
                nc.tensor.transpose(btp1[:], SIG0b[:, 0:128], ID)
                nc.tensor.transpose(btp3[:], SIG0b[:, 128:192], ID)
                nc.tensor.transpose(btp2[:], SIG1b[:, 0:128], ID[0:64, 0:64])
                nc.tensor.transpose(btp4[:], SIG1b[:, 128:192], ID[0:64, 0:64])
                nc.vector.tensor_copy(SIGT0[:, 0:128], btp1[:])
                nc.vector.tensor_copy(SIGT1x[0:64, 0:128], btp3[:])
                nc.scalar.copy(SIGT0[:, 128:192], btp2[:])
                nc.vector.tensor_copy(SIGT1x[0:64, 128:192], btp4[:])
            else:
                o0 = out_p.tile([P, S], f32, tag="o0")
                o1 = out_p.tile([64, S], f32, tag="o1")
                nc.scalar.activation(o1[:], qp1[:], Sig)
                nc.gpsimd.dma_start(qout_d.ap()[128:192, :], o1[:])
                nc.scalar.activation(o0[:], qp0[:], Sig)
                nc.sync.dma_start(qout_d.ap()[0:128, :], o0[:])
    nc.compile()
    return nc


def _get_program():
    if "nc" not in _CACHE:
        _CACHE["nc"] = _build_program()
    return _CACHE["nc"]


def _prep_core_inputs(s_con_b, sbm16_b):
    """Per-batch input dict. sbm16_b: masked s_bin, fp16, [i, j, k]."""
    T = sbm16_b.transpose(2, 1, 0)                  # [k, j, i]
    sconT = np.ascontiguousarray(s_con_b.T).astype(np.float16)   # [j, i]
    sig1T = (1.0 / (1.0 + np.exp(-s_con_b))).astype(np.float16).T  # [k, j]
    c0 = np.concatenate(
        [np.eye(P, dtype=np.float16),
         np.ascontiguousarray(sig1T[0:128]),
         np.ascontiguousarray(T[0:128]).reshape(P, SS)], 1)
    c1x = np.concatenate(
        [np.concatenate([np.ascontiguousarray(sig1T[128:192]),
                         np.ones((1, S), dtype=np.float16)], 0),
         np.concatenate([np.ascontiguousarray(T[128:192]).reshape(64, SS),
                         sconT.reshape(1, SS)], 0)], 1)
    return {"c0": c0, "c1x": c1x}


def kernel(s_con, s_bin, mask):
    from concourse.bass_utils import run_bass_kernel_spmd

    s_con = np.asarray(s_con, dtype=np.float32)
    s_bin = np.asarray(s_bin, dtype=np.float32)
    mask = np.asarray(mask)

    idx = np.arange(S)
    ne = idx[:, None] != idx[None, :]                       # [a, k]
    m2 = ne[:, None, :] & ne[None, :, :]                    # [i, j, k]
    full_mask = mask[:, :, :, None] & m2[None]              # [B, i, j, k]
    sbm16 = (s_bin * full_mask).astype(np.float16)

    nc = _get_program()
    in_maps = [_prep_core_inputs(s_con[b], sbm16[b]) for b in range(B)]
    res = run_bass_kernel_spmd(nc, in_maps, list(range(B)))
    out = np.stack([res.results[b]["qout"] for b in range(B)], 0)
    return np.ascontiguousarray(out.astype(np.float32))
